# revision 17
# baseline (speedup 1.0000x reference)
"""Trainium2 Bass kernel for single-head causal attention.

Problem: nn_AttentionHead (B=4, T=4096, E=1024, DH=128), fp32 inputs/outputs.

  q = x @ Wq; k = x @ Wk; v = x @ Wv            (per batch)
  out = softmax(causal(q k^T / sqrt(DH))) @ v

Sharding: 8 cores = 4 batches x 2 query-shards. The two cores of a batch
pair split the queries at 128-row-tile granularity, interleaved (core p
takes q-tiles p, p+2, p+4, ...) so the causal triangle load-balances.

All 8 cores run ONE uniform SPMD program: the per-core query selection and
the causal structure are folded into host-side data:
  - x rows are permuted per core so that within each window of 8 tiles the
    core's own 4 q-tiles come first ("q-half"), the other parity's after.
  - 8 additive mask tiles [128, 512] (0 / -1e30) encode causal validity of
    the last 8 key-tiles of every attention group vs the group's 4
    query-subtiles (identical for every window by construction).

On-chip dataflow per core (all matmuls in float32r: full PE rate, ~1.5e-4
matmul relative error vs 2.4e-3 for bf16):
  - PE-transpose x chunks -> X^T [e, t] (fp32, rounded to f32r at PSUM
    evacuation), project to K^T/Q^T [d, t] and V^T -> V [k, d].
  - Scores computed transposed: S^T[k, q] = (K^T)^T.T.. = matmul(lhsT=K^T
    tile, rhs=Q^T group), softmax over k (the partition dim):
      exp on ACT (no max-subtraction: scores ~ N(0,1), exp is safe),
      denominator via ones-vector matmul accumulated in PSUM,
      AV as out^T[d,q] = matmul(lhsT=V tile, rhs=P^T) -- no P transposes.
  - Final PE transpose back to [q, d], divide by denominator, DMA out.
"""

import numpy as np

import concourse.bass as bass
import concourse.mybir as mybir
import concourse.tile as tile
from concourse import bacc
from concourse.bass_utils import run_bass_kernel_spmd

P = 128
B, T, E, DH = 4, 4096, 1024, 128
ECH = E // P            # 8 e-chunks
NSC = T // (4 * P)      # 8 super-chunks of 512 rows
NG = NSC // 2           # 4 attention groups of 512 queries per core
NKT = T // P            # 32 key tiles
QG = 4 * P              # 512 queries per group
NCORES = 8
SCALE = 1.0 / np.sqrt(DH)
NEG = -1.0e30

f32 = mybir.dt.float32
f32r = mybir.dt.float32r


def _make_identity(nc, ident):
    nc.gpsimd.memset(ident, 0.0)
    nc.gpsimd.affine_select(
        out=ident, in_=ident,
        compare_op=mybir.AluOpType.not_equal,
        fill=1.0, base=0,
        pattern=[[-1, P]], channel_multiplier=1,
    )


def build_nc():
    nc = bacc.Bacc("TRN2", target_bir_lowering=False, debug=False,
                   num_devices=NCORES)
    x = nc.dram_tensor("x", [T, E], f32, kind="ExternalInput").ap()
    wq = nc.dram_tensor("wq", [E, DH], f32, kind="ExternalInput").ap()
    wk = nc.dram_tensor("wk", [E, DH], f32, kind="ExternalInput").ap()
    wv = nc.dram_tensor("wv", [E, DH], f32, kind="ExternalInput").ap()
    masks = nc.dram_tensor("masks", [8, P, QG], f32, kind="ExternalInput").ap()
    out = nc.dram_tensor("out", [T // 2, DH], f32, kind="ExternalOutput").ap()
    den_scr = nc.dram_tensor("den_scr", [NG, QG], f32, kind="Internal").ap()

    with tile.TileContext(nc) as tc:
        _emit(nc, tc, x, wq, wk, wv, masks, out, den_scr)
    nc.compile()
    return nc


def _emit(nc, tc, x, wq, wk, wv, masks, out, den_scr):
    import contextlib
    ctx = contextlib.ExitStack()
    with ctx:
        const = ctx.enter_context(tc.tile_pool(name="const", bufs=1))
        xin_pool = ctx.enter_context(tc.tile_pool(name="xin", bufs=2))
        xt_pool = ctx.enter_context(tc.tile_pool(name="xt", bufs=2))
        kv_pool = ctx.enter_context(tc.tile_pool(name="kv", bufs=1))
        vtt_pool = ctx.enter_context(tc.tile_pool(name="vtt", bufs=2))
        pt_pool = ctx.enter_context(tc.tile_pool(name="pt", bufs=4))
        osb_pool = ctx.enter_context(tc.tile_pool(name="osb", bufs=2))
        sm_pool = ctx.enter_context(tc.tile_pool(name="sm", bufs=2))
        st_psum = ctx.enter_context(
            tc.tile_pool(name="stp", bufs=1, space="PSUM"))
        scr_psum = ctx.enter_context(
            tc.tile_pool(name="scrp", bufs=2, space="PSUM"))
        avt_psum = ctx.enter_context(
            tc.tile_pool(name="avtp", bufs=1, space="PSUM"))
        den_psum = ctx.enter_context(
            tc.tile_pool(name="denp", bufs=1, space="PSUM"))

        # ---- first x chunk, issued before everything else so the PE can
        # start transposing as early as possible ----
        xin0 = xin_pool.tile([P, 4, E], f32, name="xin")
        for tc_i in range(4):
            nc.sync.dma_start(xin0[:, tc_i, :], x[128 * tc_i:128 * (tc_i + 1), :])

        # ---- constants (weights/masks DMA on the GpSimd queue so they
        # don't delay x chunks on the sync queue) ----
        ident = const.tile([P, P], f32)
        _make_identity(nc, ident)
        ident_r = const.tile([P, P], f32r)
        nc.vector.tensor_copy(ident_r[:], ident[:])
        ones_f = const.tile([P, 1], f32)
        nc.gpsimd.memset(ones_f, 1.0)
        ones_r = const.tile([P, 1], f32r)
        nc.vector.tensor_copy(ones_r[:], ones_f[:])

        w_r = {}
        for name, wap in (("wk", wk), ("wv", wv), ("wq", wq)):
            wtmp = const.tile([P, ECH, DH], f32, name=f"{name}_tmp")
            nc.gpsimd.dma_start(
                wtmp[:], wap.rearrange("(eo ei) d -> ei eo d", ei=P))
            wr = const.tile([P, ECH, DH], f32r, name=f"{name}_r")
            nc.vector.tensor_copy(wr[:], wtmp[:])
            w_r[name] = wr

        # masks are not needed until the first attention group; emitted
        # late so their DMA doesn't delay the first x chunk
        masks_sb = const.tile([P, 8, QG], f32)

        # persistent K^T [d, k], V [k; kt, d], Q^T [d; group, q]
        kt_sb = kv_pool.tile([P, NKT * P], f32r)
        v_sb = kv_pool.tile([P, NKT * DH], f32r)
        qt_sb = kv_pool.tile([P, NG * QG], f32r)

        def evac(dst, src):
            # PSUM->SBUF evacuations on VectorE (ScalarE is kept exp-only
            # so the attention softmax chain never queues behind copies)
            nc.vector.tensor_copy(dst, src)

        def prep(sc):
            if sc == 0:
                xin = xin0
            else:
                xin = xin_pool.tile([P, 4, E], f32, name="xin")
                for tc_i in range(4):
                    r0 = 512 * sc + 128 * tc_i
                    nc.sync.dma_start(xin[:, tc_i, :], x[r0:r0 + P, :])
            # X^T chunk [e(128 x 8ch), t(512)] in f32r
            xt = xt_pool.tile([P, ECH, 4 * P], f32r, name="xt")
            for tc_i in range(4):
                for half in range(2):
                    tp = scr_psum.tile([P, 4 * P], f32, tag="scr",
                                       name="tp_xt")
                    for e4 in range(4):
                        ec = 4 * half + e4
                        nc.tensor.transpose(
                            tp[:, e4 * P:(e4 + 1) * P],
                            xin[:, tc_i, ec * P:(ec + 1) * P],
                            ident[:])
                    evac(xt[:, 4 * half:4 * half + 4,
                            tc_i * P:(tc_i + 1) * P],
                         tp[:].rearrange("p (a t) -> p a t", a=4))

            # K^T chunk: [d, 512 keys]
            ktp = scr_psum.tile([P, 4 * P], f32, tag="scr", name="ktp")
            for ec in range(ECH):
                nc.tensor.matmul(ktp[:], w_r["wk"][:, ec, :], xt[:, ec, :],
                                 start=(ec == 0), stop=(ec == ECH - 1))
            evac(kt_sb[:, 512 * sc:512 * (sc + 1)], ktp[:])

            # V^T chunk then transpose to V natural [k, d]
            vtp = scr_psum.tile([P, 4 * P], f32, tag="scr", name="vtp")
            for ec in range(ECH):
                nc.tensor.matmul(vtp[:], w_r["wv"][:, ec, :], xt[:, ec, :],
                                 start=(ec == 0), stop=(ec == ECH - 1))
            vtt = vtt_pool.tile([P, 4 * P], f32r, name="vtt")
            evac(vtt[:], vtp[:])
            vnp = scr_psum.tile([P, 4 * P], f32r, tag="scr", name="vnp")
            for kb in range(4):
                nc.tensor.transpose(
                    vnp[:, kb * P:(kb + 1) * P],
                    vtt[:, kb * P:(kb + 1) * P],
                    ident_r[:])
            evac(v_sb[:, 512 * sc:512 * (sc + 1)], vnp[:])

            # Q^T: even super-chunks hold this core's queries
            if sc % 2 == 0:
                qtp = scr_psum.tile([P, 4 * P], f32, tag="scr", name="qtp")
                for ec in range(ECH):
                    nc.tensor.matmul(qtp[:], w_r["wq"][:, ec, :],
                                     xt[:, ec, :],
                                     start=(ec == 0), stop=(ec == ECH - 1))
                g = sc // 2
                evac(qt_sb[:, QG * g:QG * (g + 1)], qtp[:])

        def attn(g):
            nk = 8 * g + 8          # key tiles for this group
            npair = nk // 2
            qt_g = qt_sb[:, QG * g:QG * (g + 1)]
            avt = avt_psum.tile([P, QG], f32, name="avt")
            den = den_psum.tile([1, QG], f32, name="den")
            # 4-bank score ring: slot-level (subtile) deps let the MM for
            # pair m+1 run while exp still reads pair m's two banks
            st_ring = st_psum.tile([P, 4, QG], f32, name="st_ring")
            for m in range(npair):
                s0 = 2 * (m % 2)
                for h in range(2):
                    kt = 2 * m + h
                    nc.tensor.matmul(
                        st_ring[:, s0 + h, :],
                        kt_sb[:, kt * P:(kt + 1) * P], qt_g,
                        start=True, stop=True)
                pt = pt_pool.tile([P, 2, QG], f32r, name="pt")
                nc.scalar.activation(pt[:], st_ring[:, s0:s0 + 2, :],
                                     mybir.ActivationFunctionType.Exp,
                                     scale=SCALE)
                # multiplicative 0/1 causal mask post-exp (pair-aligned)
                if m >= npair - 4:
                    j0 = 2 * (m - (npair - 4))
                    nc.vector.tensor_mul(pt[:], pt[:],
                                         masks_sb[:, j0:j0 + 2, :])
                for h in range(2):
                    kt = 2 * m + h
                    nc.tensor.matmul(
                        avt[:], v_sb[:, kt * P:(kt + 1) * P], pt[:, h, :],
                        start=(kt == 0), stop=(kt == nk - 1))
                    nc.tensor.matmul(
                        den[:], ones_r[:], pt[:, h, :],
                        start=(kt == 0), stop=(kt == nk - 1))

            # epilogue: denominators back in natural q layout via DRAM bounce
            den_sb = sm_pool.tile([1, QG], f32, name="den_sb")
            nc.vector.tensor_copy(den_sb[:], den[:])
            nc.sync.dma_start(den_scr[g:g + 1, :], den_sb[:])
            rsrc = sm_pool.tile([P, 4], f32, name="rsrc")
            nc.sync.dma_start(
                rsrc[:], den_scr[g].rearrange("(a q) -> q a", a=4))
            recip = sm_pool.tile([P, 4], f32, name="recip")
            nc.vector.reciprocal(recip[:], rsrc[:])

            avt_sb = osb_pool.tile([P, QG], f32, name="avt_sb")
            nc.vector.tensor_copy(avt_sb[:], avt[:])
            natp = scr_psum.tile([P, QG], f32, tag="scr", name="natp")
            for a in range(4):
                nc.tensor.transpose(
                    natp[:, a * P:(a + 1) * P],
                    avt_sb[:, a * P:(a + 1) * P],
                    ident[:])
            out_sb = osb_pool.tile([P, QG], f32, name="out_sb")
            for a in range(4):
                nc.vector.tensor_scalar_mul(
                    out_sb[:, a * DH:(a + 1) * DH],
                    natp[:, a * DH:(a + 1) * DH],
                    recip[:, a:a + 1])
            nc.sync.dma_start(
                out[QG * g:QG * (g + 1), :].rearrange(
                    "(a q) d -> q a d", a=4),
                out_sb[:].rearrange("q (a d) -> q a d", a=4))

        for sc in range(NSC):
            prep(sc)
            if sc == 0:
                nc.gpsimd.dma_start(masks_sb[:],
                                    masks.rearrange("j p c -> p j c"))
            if sc % 2 == 1:
                attn((sc - 1) // 2)


# ---------------- host side ----------------

def _perm(p):
    """Permuted-tile -> true-tile index map (32 tiles of 128 rows)."""
    order = []
    for w in range(T // (8 * P)):
        order += [8 * w + p + 2 * a for a in range(4)]
        order += [8 * w + (1 - p) + 2 * a for a in range(4)]
    return np.array(order)


def _masks(p):
    """Multiplicative 0/1 causal masks [8, 128, 512] for the last 8 key
    tiles of every attention group (window-invariant by construction)."""
    m = np.zeros((8, P, QG), np.float32)
    kl = np.arange(P)[:, None]
    ql = np.arange(P)[None, :]
    tri = (kl <= ql)
    for j in range(8):
        for a in range(4):
            blk = m[j, :, a * P:(a + 1) * P]
            if j < 4:
                if j < a:
                    blk[:] = 1.0
                elif j == a:
                    blk[tri] = 1.0
            else:
                mm = j - 4
                if (p == 0 and mm < a) or (p == 1 and mm <= a):
                    blk[:] = 1.0
    return m


_NC_CACHE = []


def _get_nc():
    if not _NC_CACHE:
        _NC_CACHE.append(build_nc())
    return _NC_CACHE[0]


def _run(norm_inputs, Wq, Wk, Wv, **spmd_kwargs):
    nc = _get_nc()
    xf = np.asarray(norm_inputs, np.float32)
    wqf = np.ascontiguousarray(np.asarray(Wq, np.float32))
    wkf = np.ascontiguousarray(np.asarray(Wk, np.float32))
    wvf = np.ascontiguousarray(np.asarray(Wv, np.float32))
    in_maps = []
    for c in range(NCORES):
        b, p = c // 2, c % 2
        xp = np.ascontiguousarray(
            xf[b].reshape(NKT, P, E)[_perm(p)].reshape(T, E))
        in_maps.append({
            "x": xp, "wq": wqf, "wk": wkf, "wv": wvf,
            "masks": _masks(p),
        })
    res = run_bass_kernel_spmd(nc, in_maps, core_ids=list(range(NCORES)),
                               **spmd_kwargs)
    outf = np.empty((B, T, DH), np.float32)
    for c in range(NCORES):
        b, p = c // 2, c % 2
        oc = res.results[c]["out"].reshape(NG, 4, P, DH)
        full = outf[b].reshape(NKT, P, DH)
        for i in range(NG):
            for a in range(4):
                full[8 * i + p + 2 * a] = oc[i, a]
    return outf, res


def kernel(norm_inputs, Wq, Wk, Wv):
    outf, _ = _run(norm_inputs, Wq, Wk, Wv)
    return outf


# revision 21
# speedup vs baseline: 1.0198x; 1.0198x over previous
"""Trainium2 Bass kernel for single-head causal attention.

Problem: nn_AttentionHead (B=4, T=4096, E=1024, DH=128), fp32 inputs/outputs.

  q = x @ Wq; k = x @ Wk; v = x @ Wv            (per batch)
  out = softmax(causal(q k^T / sqrt(DH))) @ v

Sharding: 8 cores = 4 batches x 2 query-shards. The two cores of a batch
pair split the queries at 128-row-tile granularity, interleaved (core p
takes q-tiles p, p+2, p+4, ...) so the causal triangle load-balances.

All 8 cores run ONE uniform SPMD program: the per-core query selection and
the causal structure are folded into host-side data:
  - x rows are permuted per core so that within each window of 8 tiles the
    core's own 4 q-tiles come first ("q-half"), the other parity's after.
  - 8 additive mask tiles [128, 512] (0 / -1e30) encode causal validity of
    the last 8 key-tiles of every attention group vs the group's 4
    query-subtiles (identical for every window by construction).

On-chip dataflow per core (all matmuls in float32r: full PE rate, ~1.5e-4
matmul relative error vs 2.4e-3 for bf16):
  - PE-transpose x chunks -> X^T [e, t] (fp32, rounded to f32r at PSUM
    evacuation), project to K^T/Q^T [d, t] and V^T -> V [k, d].
  - Scores computed transposed: S^T[k, q] = (K^T)^T.T.. = matmul(lhsT=K^T
    tile, rhs=Q^T group), softmax over k (the partition dim):
      exp on ACT (no max-subtraction: scores ~ N(0,1), exp is safe),
      denominator via ones-vector matmul accumulated in PSUM,
      AV as out^T[d,q] = matmul(lhsT=V tile, rhs=P^T) -- no P transposes.
  - Final PE transpose back to [q, d], divide by denominator, DMA out.
"""

import numpy as np

import concourse.bass as bass
import concourse.mybir as mybir
import concourse.tile as tile
from concourse import bacc
from concourse.bass_utils import run_bass_kernel_spmd

P = 128
B, T, E, DH = 4, 4096, 1024, 128
ECH = E // P            # 8 e-chunks
NSC = T // (4 * P)      # 8 super-chunks of 512 rows
NG = NSC // 2           # 4 attention groups of 512 queries per core
NKT = T // P            # 32 key tiles
QG = 4 * P              # 512 queries per group
NCORES = 8
SCALE = 1.0 / np.sqrt(DH)
NEG = -1.0e30

f32 = mybir.dt.float32
f32r = mybir.dt.float32r


def _make_identity(nc, ident):
    nc.gpsimd.memset(ident, 0.0)
    nc.gpsimd.affine_select(
        out=ident, in_=ident,
        compare_op=mybir.AluOpType.not_equal,
        fill=1.0, base=0,
        pattern=[[-1, P]], channel_multiplier=1,
    )


def build_nc():
    nc = bacc.Bacc("TRN2", target_bir_lowering=False, debug=False,
                   num_devices=NCORES)
    x = nc.dram_tensor("x", [T, E], f32, kind="ExternalInput").ap()
    wq = nc.dram_tensor("wq", [E, DH], f32, kind="ExternalInput").ap()
    wk = nc.dram_tensor("wk", [E, DH], f32, kind="ExternalInput").ap()
    wv = nc.dram_tensor("wv", [E, DH], f32, kind="ExternalInput").ap()
    masks = nc.dram_tensor("masks", [8, P, QG], f32, kind="ExternalInput").ap()
    out = nc.dram_tensor("out", [T // 2, DH], f32, kind="ExternalOutput").ap()
    den_scr = nc.dram_tensor("den_scr", [NG, QG], f32, kind="Internal").ap()

    with tile.TileContext(nc) as tc:
        _emit(nc, tc, x, wq, wk, wv, masks, out, den_scr)
    nc.compile()
    return nc


def _emit(nc, tc, x, wq, wk, wv, masks, out, den_scr):
    import contextlib
    ctx = contextlib.ExitStack()
    with ctx:
        const = ctx.enter_context(tc.tile_pool(name="const", bufs=1))
        xin_pool = ctx.enter_context(tc.tile_pool(name="xin", bufs=2))
        xr_pool = ctx.enter_context(tc.tile_pool(name="xr", bufs=2))
        xt_pool = ctx.enter_context(tc.tile_pool(name="xt", bufs=2))
        kv_pool = ctx.enter_context(tc.tile_pool(name="kv", bufs=1))
        vtt_pool = ctx.enter_context(tc.tile_pool(name="vtt", bufs=2))
        pt_pool = ctx.enter_context(tc.tile_pool(name="pt", bufs=4))
        osb_pool = ctx.enter_context(tc.tile_pool(name="osb", bufs=2))
        sm_pool = ctx.enter_context(tc.tile_pool(name="sm", bufs=2))
        st_psum = ctx.enter_context(
            tc.tile_pool(name="stp", bufs=4, space="PSUM"))
        scr_psum = ctx.enter_context(
            tc.tile_pool(name="scrp", bufs=2, space="PSUM"))
        avt_psum = ctx.enter_context(
            tc.tile_pool(name="avtp", bufs=1, space="PSUM"))
        den_psum = ctx.enter_context(
            tc.tile_pool(name="denp", bufs=1, space="PSUM"))

        # ---- first x chunk, issued before everything else so the PE can
        # start transposing as early as possible ----
        xin0 = xin_pool.tile([P, 4, E], f32, name="xin")
        for tc_i in range(4):
            nc.sync.dma_start(xin0[:, tc_i, :], x[128 * tc_i:128 * (tc_i + 1), :])

        # ---- constants (weights/masks DMA on the GpSimd queue so they
        # don't delay x chunks on the sync queue) ----
        ident = const.tile([P, P], f32)
        _make_identity(nc, ident)
        ident_r = const.tile([P, P], f32r)
        nc.vector.tensor_copy(ident_r[:], ident[:])
        ones_f = const.tile([P, 1], f32)
        nc.gpsimd.memset(ones_f, 1.0)
        ones_r = const.tile([P, 1], f32r)
        nc.vector.tensor_copy(ones_r[:], ones_f[:])

        w_r = {}
        for name, wap in (("wk", wk), ("wv", wv), ("wq", wq)):
            wtmp = const.tile([P, ECH, DH], f32, name=f"{name}_tmp")
            nc.gpsimd.dma_start(
                wtmp[:], wap.rearrange("(eo ei) d -> ei eo d", ei=P))
            wr = const.tile([P, ECH, DH], f32r, name=f"{name}_r")
            nc.vector.tensor_copy(wr[:], wtmp[:])
            w_r[name] = wr

        # masks are not needed until the first attention group; emitted
        # late so their DMA doesn't delay the first x chunk
        masks_sb = const.tile([P, 8, QG], f32)

        # persistent K^T [d, k], V [k; kt, d], Q^T [d; group, q]
        kt_sb = kv_pool.tile([P, NKT * P], f32r)
        v_sb = kv_pool.tile([P, NKT * DH], f32r)
        qt_sb = kv_pool.tile([P, NG * QG], f32r)

        def evac(dst, src):
            # PSUM->SBUF evacuations on VectorE (ScalarE is kept exp-only
            # so the attention softmax chain never queues behind copies)
            nc.vector.tensor_copy(dst, src)

        def prep(sc):
            if sc == 0:
                xin = xin0
            else:
                xin = xin_pool.tile([P, 4, E], f32, name="xin")
                for tc_i in range(4):
                    r0 = 512 * sc + 128 * tc_i
                    nc.sync.dma_start(xin[:, tc_i, :], x[r0:r0 + P, :])
            # round x to f32r on GpSimd (otherwise idle) so the PE
            # transposes run at 1.5 cyc/row instead of fp32's 2
            xr = xr_pool.tile([P, 4, E], f32r, name="xr")
            for tc_i in range(4):
                nc.gpsimd.tensor_copy(xr[:, tc_i, :], xin[:, tc_i, :])
            # X^T chunk [e(128 x 8ch), t(512)] in f32r
            xt = xt_pool.tile([P, ECH, 4 * P], f32r, name="xt")
            for tc_i in range(4):
                for half in range(2):
                    tp = scr_psum.tile([P, 4 * P], f32r, tag="scr",
                                       name="tp_xt")
                    for e4 in range(4):
                        ec = 4 * half + e4
                        nc.tensor.transpose(
                            tp[:, e4 * P:(e4 + 1) * P],
                            xr[:, tc_i, ec * P:(ec + 1) * P],
                            ident_r[:])
                    evac(xt[:, 4 * half:4 * half + 4,
                            tc_i * P:(tc_i + 1) * P],
                         tp[:].rearrange("p (a t) -> p a t", a=4))

            # K^T chunk: [d, 512 keys]
            ktp = scr_psum.tile([P, 4 * P], f32, tag="scr", name="ktp")
            for ec in range(ECH):
                nc.tensor.matmul(ktp[:], w_r["wk"][:, ec, :], xt[:, ec, :],
                                 start=(ec == 0), stop=(ec == ECH - 1))
            evac(kt_sb[:, 512 * sc:512 * (sc + 1)], ktp[:])

            # V^T chunk then transpose to V natural [k, d]
            vtp = scr_psum.tile([P, 4 * P], f32, tag="scr", name="vtp")
            for ec in range(ECH):
                nc.tensor.matmul(vtp[:], w_r["wv"][:, ec, :], xt[:, ec, :],
                                 start=(ec == 0), stop=(ec == ECH - 1))
            vtt = vtt_pool.tile([P, 4 * P], f32r, name="vtt")
            evac(vtt[:], vtp[:])
            vnp = scr_psum.tile([P, 4 * P], f32r, tag="scr", name="vnp")
            for kb in range(4):
                nc.tensor.transpose(
                    vnp[:, kb * P:(kb + 1) * P],
                    vtt[:, kb * P:(kb + 1) * P],
                    ident_r[:])
            evac(v_sb[:, 512 * sc:512 * (sc + 1)], vnp[:])

            # Q^T: even super-chunks hold this core's queries
            if sc % 2 == 0:
                qtp = scr_psum.tile([P, 4 * P], f32, tag="scr", name="qtp")
                for ec in range(ECH):
                    nc.tensor.matmul(qtp[:], w_r["wq"][:, ec, :],
                                     xt[:, ec, :],
                                     start=(ec == 0), stop=(ec == ECH - 1))
                g = sc // 2
                evac(qt_sb[:, QG * g:QG * (g + 1)], qtp[:])

        def attn(g):
            nk = 8 * g + 8          # key tiles for this group
            qt_g = qt_sb[:, QG * g:QG * (g + 1)]
            avt = avt_psum.tile([P, QG], f32, name="avt")
            den = den_psum.tile([1, QG], f32, name="den")
            for kt in range(nk):
                st = st_psum.tile([P, QG], f32, name="st")
                nc.tensor.matmul(
                    st[:], kt_sb[:, kt * P:(kt + 1) * P], qt_g,
                    start=True, stop=True)
                pt = pt_pool.tile([P, QG], f32r, name="pt")
                nc.scalar.activation(pt[:], st[:],
                                     mybir.ActivationFunctionType.Exp,
                                     scale=SCALE)
                # multiplicative 0/1 causal mask, applied post-exp so the
                # ST-psum slot is released by exp alone
                j = kt - (nk - 8)
                if j >= 0:
                    nc.vector.tensor_mul(pt[:], pt[:], masks_sb[:, j, :])
                nc.tensor.matmul(
                    avt[:], v_sb[:, kt * P:(kt + 1) * P], pt[:],
                    start=(kt == 0), stop=(kt == nk - 1))
                nc.tensor.matmul(
                    den[:], ones_r[:], pt[:],
                    start=(kt == 0), stop=(kt == nk - 1))

            # epilogue: denominators back in natural q layout via DRAM bounce
            den_sb = sm_pool.tile([1, QG], f32, name="den_sb")
            nc.vector.tensor_copy(den_sb[:], den[:])
            nc.sync.dma_start(den_scr[g:g + 1, :], den_sb[:])
            rsrc = sm_pool.tile([P, 4], f32, name="rsrc")
            nc.sync.dma_start(
                rsrc[:], den_scr[g].rearrange("(a q) -> q a", a=4))
            recip = sm_pool.tile([P, 4], f32, name="recip")
            nc.vector.reciprocal(recip[:], rsrc[:])

            avt_sb = osb_pool.tile([P, QG], f32, name="avt_sb")
            nc.vector.tensor_copy(avt_sb[:], avt[:])
            natp = scr_psum.tile([P, QG], f32, tag="scr", name="natp")
            for a in range(4):
                nc.tensor.transpose(
                    natp[:, a * P:(a + 1) * P],
                    avt_sb[:, a * P:(a + 1) * P],
                    ident[:])
            out_sb = osb_pool.tile([P, QG], f32, name="out_sb")
            for a in range(4):
                nc.vector.tensor_scalar_mul(
                    out_sb[:, a * DH:(a + 1) * DH],
                    natp[:, a * DH:(a + 1) * DH],
                    recip[:, a:a + 1])
            nc.sync.dma_start(
                out[QG * g:QG * (g + 1), :].rearrange(
                    "(a q) d -> q a d", a=4),
                out_sb[:].rearrange("q (a d) -> q a d", a=4))

        for sc in range(NSC):
            prep(sc)
            if sc == 0:
                nc.gpsimd.dma_start(masks_sb[:],
                                    masks.rearrange("j p c -> p j c"))
            if sc % 2 == 1:
                attn((sc - 1) // 2)


# ---------------- host side ----------------

def _perm(p):
    """Permuted-tile -> true-tile index map (32 tiles of 128 rows)."""
    order = []
    for w in range(T // (8 * P)):
        order += [8 * w + p + 2 * a for a in range(4)]
        order += [8 * w + (1 - p) + 2 * a for a in range(4)]
    return np.array(order)


def _masks(p):
    """Multiplicative 0/1 causal masks [8, 128, 512] for the last 8 key
    tiles of every attention group (window-invariant by construction)."""
    m = np.zeros((8, P, QG), np.float32)
    kl = np.arange(P)[:, None]
    ql = np.arange(P)[None, :]
    tri = (kl <= ql)
    for j in range(8):
        for a in range(4):
            blk = m[j, :, a * P:(a + 1) * P]
            if j < 4:
                if j < a:
                    blk[:] = 1.0
                elif j == a:
                    blk[tri] = 1.0
            else:
                mm = j - 4
                if (p == 0 and mm < a) or (p == 1 and mm <= a):
                    blk[:] = 1.0
    return m


_NC_CACHE = []


def _get_nc():
    if not _NC_CACHE:
        _NC_CACHE.append(build_nc())
    return _NC_CACHE[0]


def _run(norm_inputs, Wq, Wk, Wv, **spmd_kwargs):
    nc = _get_nc()
    xf = np.asarray(norm_inputs, np.float32)
    wqf = np.ascontiguousarray(np.asarray(Wq, np.float32))
    wkf = np.ascontiguousarray(np.asarray(Wk, np.float32))
    wvf = np.ascontiguousarray(np.asarray(Wv, np.float32))
    in_maps = []
    for c in range(NCORES):
        b, p = c // 2, c % 2
        xp = np.ascontiguousarray(
            xf[b].reshape(NKT, P, E)[_perm(p)].reshape(T, E))
        in_maps.append({
            "x": xp, "wq": wqf, "wk": wkf, "wv": wvf,
            "masks": _masks(p),
        })
    res = run_bass_kernel_spmd(nc, in_maps, core_ids=list(range(NCORES)),
                               **spmd_kwargs)
    outf = np.empty((B, T, DH), np.float32)
    for c in range(NCORES):
        b, p = c // 2, c % 2
        oc = res.results[c]["out"].reshape(NG, 4, P, DH)
        full = outf[b].reshape(NKT, P, DH)
        for i in range(NG):
            for a in range(4):
                full[8 * i + p + 2 * a] = oc[i, a]
    return outf, res


def kernel(norm_inputs, Wq, Wk, Wv):
    outf, _ = _run(norm_inputs, Wq, Wk, Wv)
    return outf


# revision 23
# speedup vs baseline: 1.2064x; 1.1830x over previous
"""Trainium2 Bass kernel for single-head causal attention.

Problem: nn_AttentionHead (B=4, T=4096, E=1024, DH=128), fp32 inputs/outputs.

  q = x @ Wq; k = x @ Wk; v = x @ Wv            (per batch)
  out = softmax(causal(q k^T / sqrt(DH))) @ v

Sharding: 8 cores = 4 batches x 2 query-shards. The two cores of a batch
pair split the queries at 128-row-tile granularity, interleaved (core p
takes q-tiles p, p+2, p+4, ...) so the causal triangle load-balances.

All 8 cores run ONE uniform SPMD program: the per-core query selection and
the causal structure are folded into host-side data:
  - x rows are permuted per core so that within each window of 8 tiles the
    core's own 4 q-tiles come first ("q-half"), the other parity's after.
  - 8 additive mask tiles [128, 512] (0 / -1e30) encode causal validity of
    the last 8 key-tiles of every attention group vs the group's 4
    query-subtiles (identical for every window by construction).

On-chip dataflow per core (all matmuls in float32r: full PE rate, ~1.5e-4
matmul relative error vs 2.4e-3 for bf16):
  - PE-transpose x chunks -> X^T [e, t] (fp32, rounded to f32r at PSUM
    evacuation), project to K^T/Q^T [d, t] and V^T -> V [k, d].
  - Scores computed transposed: S^T[k, q] = (K^T)^T.T.. = matmul(lhsT=K^T
    tile, rhs=Q^T group), softmax over k (the partition dim):
      exp on ACT (no max-subtraction: scores ~ N(0,1), exp is safe),
      denominator via ones-vector matmul accumulated in PSUM,
      AV as out^T[d,q] = matmul(lhsT=V tile, rhs=P^T) -- no P transposes.
  - Final PE transpose back to [q, d], divide by denominator, DMA out.
"""

import numpy as np

import concourse.bass as bass
import concourse.mybir as mybir
import concourse.tile as tile
from concourse import bacc
from concourse.bass_utils import run_bass_kernel_spmd

P = 128
B, T, E, DH = 4, 4096, 1024, 128
ECH = E // P            # 8 e-chunks
NSC = T // (4 * P)      # 8 super-chunks of 512 rows
NG = NSC // 2           # 4 attention groups of 512 queries per core
NKT = T // P            # 32 key tiles
QG = 4 * P              # 512 queries per group
NCORES = 8
SCALE = 1.0 / np.sqrt(DH)
NEG = -1.0e30

f32 = mybir.dt.float32
f32r = mybir.dt.float32r


def _make_identity(nc, ident):
    nc.gpsimd.memset(ident, 0.0)
    nc.gpsimd.affine_select(
        out=ident, in_=ident,
        compare_op=mybir.AluOpType.not_equal,
        fill=1.0, base=0,
        pattern=[[-1, P]], channel_multiplier=1,
    )


def build_nc():
    nc = bacc.Bacc("TRN2", target_bir_lowering=False, debug=False,
                   num_devices=NCORES)
    x = nc.dram_tensor("x", [T, E], f32, kind="ExternalInput").ap()
    wq = nc.dram_tensor("wq", [E, DH], f32, kind="ExternalInput").ap()
    wk = nc.dram_tensor("wk", [E, DH], f32, kind="ExternalInput").ap()
    wv = nc.dram_tensor("wv", [E, DH], f32, kind="ExternalInput").ap()
    masks = nc.dram_tensor("masks", [8, P, QG], f32, kind="ExternalInput").ap()
    out = nc.dram_tensor("out", [T // 2, DH], f32, kind="ExternalOutput").ap()
    den_scr = nc.dram_tensor("den_scr", [NG, QG], f32, kind="Internal").ap()

    with tile.TileContext(nc) as tc:
        _emit(nc, tc, x, wq, wk, wv, masks, out, den_scr)
    nc.compile()
    return nc


def _emit(nc, tc, x, wq, wk, wv, masks, out, den_scr):
    import contextlib
    ctx = contextlib.ExitStack()
    with ctx:
        const = ctx.enter_context(tc.tile_pool(name="const", bufs=1))
        xin_pool = ctx.enter_context(tc.tile_pool(name="xin", bufs=2))
        xt_pool = ctx.enter_context(tc.tile_pool(name="xt", bufs=2))
        kv_pool = ctx.enter_context(tc.tile_pool(name="kv", bufs=1))
        vtt_pool = ctx.enter_context(tc.tile_pool(name="vtt", bufs=2))
        pt_pool = ctx.enter_context(tc.tile_pool(name="pt", bufs=4))
        osb_pool = ctx.enter_context(tc.tile_pool(name="osb", bufs=2))
        sm_pool = ctx.enter_context(tc.tile_pool(name="sm", bufs=2))
        st_psum = ctx.enter_context(
            tc.tile_pool(name="stp", bufs=4, space="PSUM"))
        scr_psum = ctx.enter_context(
            tc.tile_pool(name="scrp", bufs=2, space="PSUM"))
        avt_psum = ctx.enter_context(
            tc.tile_pool(name="avtp", bufs=1, space="PSUM"))
        den_psum = ctx.enter_context(
            tc.tile_pool(name="denp", bufs=1, space="PSUM"))

        # ---- first x chunk, issued before everything else so the PE can
        # start transposing as early as possible ----
        xin0 = xin_pool.tile([P, 4, E], f32, name="xin")
        for tc_i in range(4):
            nc.sync.dma_start(xin0[:, tc_i, :], x[128 * tc_i:128 * (tc_i + 1), :])

        # ---- constants (weights/masks DMA on the GpSimd queue so they
        # don't delay x chunks on the sync queue) ----
        ident = const.tile([P, P], f32)
        _make_identity(nc, ident)
        ident_r = const.tile([P, P], f32r)
        nc.vector.tensor_copy(ident_r[:], ident[:])
        ones_f = const.tile([P, 1], f32)
        nc.gpsimd.memset(ones_f, 1.0)
        ones_r = const.tile([P, 1], f32r)
        nc.vector.tensor_copy(ones_r[:], ones_f[:])

        w_r = {}
        for name, wap in (("wk", wk), ("wv", wv), ("wq", wq)):
            wtmp = const.tile([P, ECH, DH], f32, name=f"{name}_tmp")
            nc.gpsimd.dma_start(
                wtmp[:], wap.rearrange("(eo ei) d -> ei eo d", ei=P))
            wr = const.tile([P, ECH, DH], f32r, name=f"{name}_r")
            nc.vector.tensor_copy(wr[:], wtmp[:])
            w_r[name] = wr

        # masks are not needed until the first attention group; emitted
        # late so their DMA doesn't delay the first x chunk
        masks_sb = const.tile([P, 8, QG], f32)

        # persistent K^T [d, k], V [k; kt, d], Q^T [d; group, q]
        kt_sb = kv_pool.tile([P, NKT * P], f32r)
        v_sb = kv_pool.tile([P, NKT * DH], f32r)
        qt_sb = kv_pool.tile([P, NG * QG], f32r)

        def evac(dst, src):
            # PSUM->SBUF evacuations on VectorE (ScalarE is kept exp-only
            # so the attention softmax chain never queues behind copies)
            nc.vector.tensor_copy(dst, src)

        def prep(sc):
            if sc == 0:
                xin = xin0
            else:
                xin = xin_pool.tile([P, 4, E], f32, name="xin")
                for tc_i in range(4):
                    r0 = 512 * sc + 128 * tc_i
                    nc.sync.dma_start(xin[:, tc_i, :], x[r0:r0 + P, :])
            # X^T chunk [e(128 x 8ch), t(512)] in f32r
            xt = xt_pool.tile([P, ECH, 4 * P], f32r, name="xt")
            for tc_i in range(4):
                for half in range(2):
                    tp = scr_psum.tile([P, 4 * P], f32, tag="scr",
                                       name="tp_xt")
                    for e4 in range(4):
                        ec = 4 * half + e4
                        nc.tensor.transpose(
                            tp[:, e4 * P:(e4 + 1) * P],
                            xin[:, tc_i, ec * P:(ec + 1) * P],
                            ident[:])
                    evac(xt[:, 4 * half:4 * half + 4,
                            tc_i * P:(tc_i + 1) * P],
                         tp[:].rearrange("p (a t) -> p a t", a=4))

            # K^T chunk: [d, 512 keys]
            ktp = scr_psum.tile([P, 4 * P], f32, tag="scr", name="ktp")
            for ec in range(ECH):
                nc.tensor.matmul(ktp[:], w_r["wk"][:, ec, :], xt[:, ec, :],
                                 start=(ec == 0), stop=(ec == ECH - 1))
            evac(kt_sb[:, 512 * sc:512 * (sc + 1)], ktp[:])

            # V^T chunk then transpose to V natural [k, d]
            vtp = scr_psum.tile([P, 4 * P], f32, tag="scr", name="vtp")
            for ec in range(ECH):
                nc.tensor.matmul(vtp[:], w_r["wv"][:, ec, :], xt[:, ec, :],
                                 start=(ec == 0), stop=(ec == ECH - 1))
            vtt = vtt_pool.tile([P, 4 * P], f32r, name="vtt")
            evac(vtt[:], vtp[:])
            vnp = scr_psum.tile([P, 4 * P], f32r, tag="scr", name="vnp")
            for kb in range(4):
                nc.tensor.transpose(
                    vnp[:, kb * P:(kb + 1) * P],
                    vtt[:, kb * P:(kb + 1) * P],
                    ident_r[:])
            evac(v_sb[:, 512 * sc:512 * (sc + 1)], vnp[:])

            # Q^T: even super-chunks hold this core's queries
            if sc % 2 == 0:
                qtp = scr_psum.tile([P, 4 * P], f32, tag="scr", name="qtp")
                for ec in range(ECH):
                    nc.tensor.matmul(qtp[:], w_r["wq"][:, ec, :],
                                     xt[:, ec, :],
                                     start=(ec == 0), stop=(ec == ECH - 1))
                g = sc // 2
                evac(qt_sb[:, QG * g:QG * (g + 1)], qtp[:])

        def attn(g):
            nk = 8 * g + 8          # key tiles for this group
            qt_g = qt_sb[:, QG * g:QG * (g + 1)]
            avt = avt_psum.tile([P, QG], f32, name="avt")
            den = den_psum.tile([1, QG], f32, name="den")
            for kt in range(nk):
                st = st_psum.tile([P, QG], f32, name="st")
                nc.tensor.matmul(
                    st[:], kt_sb[:, kt * P:(kt + 1) * P], qt_g,
                    start=True, stop=True)
                pt = pt_pool.tile([P, QG], f32r, name="pt")
                nc.scalar.activation(pt[:], st[:],
                                     mybir.ActivationFunctionType.Exp,
                                     scale=SCALE)
                # multiplicative 0/1 causal mask, applied post-exp so the
                # ST-psum slot is released by exp alone
                j = kt - (nk - 8)
                if j >= 0:
                    nc.vector.tensor_mul(pt[:], pt[:], masks_sb[:, j, :])
                nc.tensor.matmul(
                    avt[:], v_sb[:, kt * P:(kt + 1) * P], pt[:],
                    start=(kt == 0), stop=(kt == nk - 1))
                nc.tensor.matmul(
                    den[:], ones_r[:], pt[:],
                    start=(kt == 0), stop=(kt == nk - 1))

            # epilogue: denominators back in natural q layout via DRAM bounce
            den_sb = sm_pool.tile([1, QG], f32, name="den_sb")
            nc.vector.tensor_copy(den_sb[:], den[:])
            nc.sync.dma_start(den_scr[g:g + 1, :], den_sb[:])
            rsrc = sm_pool.tile([P, 4], f32, name="rsrc")
            nc.sync.dma_start(
                rsrc[:], den_scr[g].rearrange("(a q) -> q a", a=4))
            recip = sm_pool.tile([P, 4], f32, name="recip")
            nc.vector.reciprocal(recip[:], rsrc[:])

            avt_sb = osb_pool.tile([P, QG], f32, name="avt_sb")
            nc.vector.tensor_copy(avt_sb[:], avt[:])
            natp = scr_psum.tile([P, QG], f32, tag="scr", name="natp")
            for a in range(4):
                nc.tensor.transpose(
                    natp[:, a * P:(a + 1) * P],
                    avt_sb[:, a * P:(a + 1) * P],
                    ident[:])
            out_sb = osb_pool.tile([P, QG], f32, name="out_sb")
            for a in range(4):
                nc.vector.tensor_scalar_mul(
                    out_sb[:, a * DH:(a + 1) * DH],
                    natp[:, a * DH:(a + 1) * DH],
                    recip[:, a:a + 1])
            nc.sync.dma_start(
                out[QG * g:QG * (g + 1), :].rearrange(
                    "(a q) d -> q a d", a=4),
                out_sb[:].rearrange("q (a d) -> q a d", a=4))

        for sc in range(NSC):
            prep(sc)
            if sc == 0:
                nc.gpsimd.dma_start(masks_sb[:],
                                    masks.rearrange("j p c -> p j c"))
            if sc % 2 == 1:
                attn((sc - 1) // 2)


# ---------------- host side ----------------

def _perm(p):
    """Permuted-tile -> true-tile index map (32 tiles of 128 rows)."""
    order = []
    for w in range(T // (8 * P)):
        order += [8 * w + p + 2 * a for a in range(4)]
        order += [8 * w + (1 - p) + 2 * a for a in range(4)]
    return np.array(order)


def _masks(p):
    """Multiplicative 0/1 causal masks [8, 128, 512] for the last 8 key
    tiles of every attention group (window-invariant by construction)."""
    m = np.zeros((8, P, QG), np.float32)
    kl = np.arange(P)[:, None]
    ql = np.arange(P)[None, :]
    tri = (kl <= ql)
    for j in range(8):
        for a in range(4):
            blk = m[j, :, a * P:(a + 1) * P]
            if j < 4:
                if j < a:
                    blk[:] = 1.0
                elif j == a:
                    blk[tri] = 1.0
            else:
                mm = j - 4
                if (p == 0 and mm < a) or (p == 1 and mm <= a):
                    blk[:] = 1.0
    return m


_NC_CACHE = []


def _get_nc():
    if not _NC_CACHE:
        _NC_CACHE.append(build_nc())
    return _NC_CACHE[0]


def _run(norm_inputs, Wq, Wk, Wv, **spmd_kwargs):
    nc = _get_nc()
    xf = np.asarray(norm_inputs, np.float32)
    wqf = np.ascontiguousarray(np.asarray(Wq, np.float32))
    wkf = np.ascontiguousarray(np.asarray(Wk, np.float32))
    wvf = np.ascontiguousarray(np.asarray(Wv, np.float32))
    in_maps = []
    for c in range(NCORES):
        b, p = c // 2, c % 2
        xp = np.ascontiguousarray(
            xf[b].reshape(NKT, P, E)[_perm(p)].reshape(T, E))
        in_maps.append({
            "x": xp, "wq": wqf, "wk": wkf, "wv": wvf,
            "masks": _masks(p),
        })
    res = run_bass_kernel_spmd(nc, in_maps, core_ids=list(range(NCORES)),
                               **spmd_kwargs)
    outf = np.empty((B, T, DH), np.float32)
    for c in range(NCORES):
        b, p = c // 2, c % 2
        oc = res.results[c]["out"].reshape(NG, 4, P, DH)
        full = outf[b].reshape(NKT, P, DH)
        for i in range(NG):
            for a in range(4):
                full[8 * i + p + 2 * a] = oc[i, a]
    return outf, res


def kernel(norm_inputs, Wq, Wk, Wv):
    outf, _ = _run(norm_inputs, Wq, Wk, Wv)
    return outf


# revision 25
# speedup vs baseline: 1.2257x; 1.0160x over previous
"""Trainium2 Bass kernel for single-head causal attention.

Problem: nn_AttentionHead (B=4, T=4096, E=1024, DH=128), fp32 inputs/outputs.

  q = x @ Wq; k = x @ Wk; v = x @ Wv            (per batch)
  out = softmax(causal(q k^T / sqrt(DH))) @ v

Sharding: 8 cores = 4 batches x 2 query-shards. The two cores of a batch
pair split the queries at 128-row-tile granularity, interleaved (core p
takes q-tiles p, p+2, p+4, ...) so the causal triangle load-balances.

All 8 cores run ONE uniform SPMD program: the per-core query selection and
the causal structure are folded into host-side data:
  - x rows are permuted per core so that within each window of 8 tiles the
    core's own 4 q-tiles come first ("q-half"), the other parity's after.
  - 8 additive mask tiles [128, 512] (0 / -1e30) encode causal validity of
    the last 8 key-tiles of every attention group vs the group's 4
    query-subtiles (identical for every window by construction).

On-chip dataflow per core (all matmuls in float32r: full PE rate, ~1.5e-4
matmul relative error vs 2.4e-3 for bf16):
  - PE-transpose x chunks -> X^T [e, t] (fp32, rounded to f32r at PSUM
    evacuation), project to K^T/Q^T [d, t] and V^T -> V [k, d].
  - Scores computed transposed: S^T[k, q] = (K^T)^T.T.. = matmul(lhsT=K^T
    tile, rhs=Q^T group), softmax over k (the partition dim):
      exp on ACT (no max-subtraction: scores ~ N(0,1), exp is safe),
      denominator via ones-vector matmul accumulated in PSUM,
      AV as out^T[d,q] = matmul(lhsT=V tile, rhs=P^T) -- no P transposes.
  - Final PE transpose back to [q, d], divide by denominator, DMA out.
"""

import numpy as np

import concourse.bass as bass
import concourse.mybir as mybir
import concourse.tile as tile
from concourse import bacc
from concourse.bass_utils import run_bass_kernel_spmd

P = 128
B, T, E, DH = 4, 4096, 1024, 128
ECH = E // P            # 8 e-chunks
NSC = T // (4 * P)      # 8 super-chunks of 512 rows
NG = NSC // 2           # 4 attention groups of 512 queries per core
NKT = T // P            # 32 key tiles
QG = 4 * P              # 512 queries per group
NCORES = 8
SCALE = 1.0 / np.sqrt(DH)
NEG = -1.0e30

f32 = mybir.dt.float32
f32r = mybir.dt.float32r


def _make_identity(nc, ident):
    nc.gpsimd.memset(ident, 0.0)
    nc.gpsimd.affine_select(
        out=ident, in_=ident,
        compare_op=mybir.AluOpType.not_equal,
        fill=1.0, base=0,
        pattern=[[-1, P]], channel_multiplier=1,
    )


def build_nc():
    nc = bacc.Bacc("TRN2", target_bir_lowering=False, debug=False,
                   num_devices=NCORES)
    x = nc.dram_tensor("x", [T, E], f32, kind="ExternalInput").ap()
    wq = nc.dram_tensor("wq", [E, DH], f32, kind="ExternalInput").ap()
    wk = nc.dram_tensor("wk", [E, DH], f32, kind="ExternalInput").ap()
    wv = nc.dram_tensor("wv", [E, DH], f32, kind="ExternalInput").ap()
    masks = nc.dram_tensor("masks", [8, P, QG], f32, kind="ExternalInput").ap()
    out = nc.dram_tensor("out", [T // 2, DH], f32, kind="ExternalOutput").ap()
    den_scr = nc.dram_tensor("den_scr", [NG, QG], f32, kind="Internal").ap()

    with tile.TileContext(nc) as tc:
        _emit(nc, tc, x, wq, wk, wv, masks, out, den_scr)
    nc.compile()
    return nc


def _emit(nc, tc, x, wq, wk, wv, masks, out, den_scr):
    import contextlib
    ctx = contextlib.ExitStack()
    with ctx:
        const = ctx.enter_context(tc.tile_pool(name="const", bufs=1))
        xin_pool = ctx.enter_context(tc.tile_pool(name="xin", bufs=2))
        xt_pool = ctx.enter_context(tc.tile_pool(name="xt", bufs=2))
        kv_pool = ctx.enter_context(tc.tile_pool(name="kv", bufs=1))
        vtt_pool = ctx.enter_context(tc.tile_pool(name="vtt", bufs=2))
        pt_pool = ctx.enter_context(tc.tile_pool(name="pt", bufs=4))
        osb_pool = ctx.enter_context(tc.tile_pool(name="osb", bufs=2))
        sm_pool = ctx.enter_context(tc.tile_pool(name="sm", bufs=2))
        st_psum = ctx.enter_context(
            tc.tile_pool(name="stp", bufs=4, space="PSUM"))
        scr_psum = ctx.enter_context(
            tc.tile_pool(name="scrp", bufs=2, space="PSUM"))
        avt_psum = ctx.enter_context(
            tc.tile_pool(name="avtp", bufs=1, space="PSUM"))
        den_psum = ctx.enter_context(
            tc.tile_pool(name="denp", bufs=1, space="PSUM"))

        # ---- first x chunk, issued before everything else so the PE can
        # start transposing as early as possible ----
        xin0 = xin_pool.tile([P, 4, E], f32, name="xin")
        for tc_i in range(4):
            nc.sync.dma_start(xin0[:, tc_i, :], x[128 * tc_i:128 * (tc_i + 1), :])

        # ---- constants (weights/masks DMA on the GpSimd queue so they
        # don't delay x chunks on the sync queue) ----
        ident = const.tile([P, P], f32)
        _make_identity(nc, ident)
        ident_r = const.tile([P, P], f32r)
        nc.vector.tensor_copy(ident_r[:], ident[:])
        ones_f = const.tile([P, 1], f32)
        nc.gpsimd.memset(ones_f, 1.0)
        ones_r = const.tile([P, 1], f32r)
        nc.vector.tensor_copy(ones_r[:], ones_f[:])

        # PE warmup: dummy matmuls during the initial DMA wait keep the PE
        # busy so the HAM clock-gate opens to 8/8 before real work starts
        warm = avt_psum.tile([P, QG], f32, tag="avt", name="warm")
        for _ in range(48):
            nc.tensor.matmul(warm[:, :P], ident_r[:], ident_r[:],
                             start=True, stop=True)

        w_r = {}
        for name, wap in (("wk", wk), ("wv", wv), ("wq", wq)):
            wtmp = const.tile([P, ECH, DH], f32, name=f"{name}_tmp")
            nc.gpsimd.dma_start(
                wtmp[:], wap.rearrange("(eo ei) d -> ei eo d", ei=P))
            wr = const.tile([P, ECH, DH], f32r, name=f"{name}_r")
            nc.vector.tensor_copy(wr[:], wtmp[:])
            w_r[name] = wr

        # masks are not needed until the first attention group; emitted
        # late so their DMA doesn't delay the first x chunk
        masks_sb = const.tile([P, 8, QG], f32)

        # persistent K^T [d, k], V [k; kt, d], Q^T [d; group, q]
        kt_sb = kv_pool.tile([P, NKT * P], f32r)
        v_sb = kv_pool.tile([P, NKT * DH], f32r)
        qt_sb = kv_pool.tile([P, NG * QG], f32r)

        def evac(dst, src):
            # PSUM->SBUF evacuations on VectorE (ScalarE is kept exp-only
            # so the attention softmax chain never queues behind copies)
            nc.vector.tensor_copy(dst, src)

        def prep(sc):
            if sc == 0:
                xin = xin0
            else:
                xin = xin_pool.tile([P, 4, E], f32, name="xin")
                for tc_i in range(4):
                    r0 = 512 * sc + 128 * tc_i
                    nc.sync.dma_start(xin[:, tc_i, :], x[r0:r0 + P, :])
            # X^T chunk [e(128 x 8ch), t(512)] in f32r
            xt = xt_pool.tile([P, ECH, 4 * P], f32r, name="xt")
            for tc_i in range(4):
                for half in range(2):
                    tp = scr_psum.tile([P, 4 * P], f32, tag="scr",
                                       name="tp_xt")
                    for e4 in range(4):
                        ec = 4 * half + e4
                        nc.tensor.transpose(
                            tp[:, e4 * P:(e4 + 1) * P],
                            xin[:, tc_i, ec * P:(ec + 1) * P],
                            ident[:])
                    evac(xt[:, 4 * half:4 * half + 4,
                            tc_i * P:(tc_i + 1) * P],
                         tp[:].rearrange("p (a t) -> p a t", a=4))

            # K^T chunk: [d, 512 keys]
            ktp = scr_psum.tile([P, 4 * P], f32, tag="scr", name="ktp")
            for ec in range(ECH):
                nc.tensor.matmul(ktp[:], w_r["wk"][:, ec, :], xt[:, ec, :],
                                 start=(ec == 0), stop=(ec == ECH - 1))
            evac(kt_sb[:, 512 * sc:512 * (sc + 1)], ktp[:])

            # V^T chunk then transpose to V natural [k, d]
            vtp = scr_psum.tile([P, 4 * P], f32, tag="scr", name="vtp")
            for ec in range(ECH):
                nc.tensor.matmul(vtp[:], w_r["wv"][:, ec, :], xt[:, ec, :],
                                 start=(ec == 0), stop=(ec == ECH - 1))
            vtt = vtt_pool.tile([P, 4 * P], f32r, name="vtt")
            evac(vtt[:], vtp[:])
            vnp = scr_psum.tile([P, 4 * P], f32r, tag="scr", name="vnp")
            for kb in range(4):
                nc.tensor.transpose(
                    vnp[:, kb * P:(kb + 1) * P],
                    vtt[:, kb * P:(kb + 1) * P],
                    ident_r[:])
            evac(v_sb[:, 512 * sc:512 * (sc + 1)], vnp[:])

            # Q^T: even super-chunks hold this core's queries
            if sc % 2 == 0:
                qtp = scr_psum.tile([P, 4 * P], f32, tag="scr", name="qtp")
                for ec in range(ECH):
                    nc.tensor.matmul(qtp[:], w_r["wq"][:, ec, :],
                                     xt[:, ec, :],
                                     start=(ec == 0), stop=(ec == ECH - 1))
                g = sc // 2
                evac(qt_sb[:, QG * g:QG * (g + 1)], qtp[:])

        def attn(g):
            nk = 8 * g + 8          # key tiles for this group
            qt_g = qt_sb[:, QG * g:QG * (g + 1)]
            avt = avt_psum.tile([P, QG], f32, tag="avt", name="avt")
            den = den_psum.tile([1, QG], f32, name="den")
            for kt in range(nk):
                st = st_psum.tile([P, QG], f32, name="st")
                nc.tensor.matmul(
                    st[:], kt_sb[:, kt * P:(kt + 1) * P], qt_g,
                    start=True, stop=True)
                pt = pt_pool.tile([P, QG], f32r, name="pt")
                nc.scalar.activation(pt[:], st[:],
                                     mybir.ActivationFunctionType.Exp,
                                     scale=SCALE)
                # multiplicative 0/1 causal mask, applied post-exp so the
                # ST-psum slot is released by exp alone
                j = kt - (nk - 8)
                if j >= 0:
                    nc.vector.tensor_mul(pt[:], pt[:], masks_sb[:, j, :])
                nc.tensor.matmul(
                    avt[:], v_sb[:, kt * P:(kt + 1) * P], pt[:],
                    start=(kt == 0), stop=(kt == nk - 1))
                nc.tensor.matmul(
                    den[:], ones_r[:], pt[:],
                    start=(kt == 0), stop=(kt == nk - 1))

            # epilogue: denominators back in natural q layout via DRAM bounce
            den_sb = sm_pool.tile([1, QG], f32, name="den_sb")
            nc.vector.tensor_copy(den_sb[:], den[:])
            nc.sync.dma_start(den_scr[g:g + 1, :], den_sb[:])
            rsrc = sm_pool.tile([P, 4], f32, name="rsrc")
            nc.sync.dma_start(
                rsrc[:], den_scr[g].rearrange("(a q) -> q a", a=4))
            recip = sm_pool.tile([P, 4], f32, name="recip")
            nc.vector.reciprocal(recip[:], rsrc[:])

            avt_sb = osb_pool.tile([P, QG], f32, name="avt_sb")
            nc.vector.tensor_copy(avt_sb[:], avt[:])
            natp = scr_psum.tile([P, QG], f32, tag="scr", name="natp")
            for a in range(4):
                nc.tensor.transpose(
                    natp[:, a * P:(a + 1) * P],
                    avt_sb[:, a * P:(a + 1) * P],
                    ident[:])
            out_sb = osb_pool.tile([P, QG], f32, name="out_sb")
            for a in range(4):
                nc.vector.tensor_scalar_mul(
                    out_sb[:, a * DH:(a + 1) * DH],
                    natp[:, a * DH:(a + 1) * DH],
                    recip[:, a:a + 1])
            nc.sync.dma_start(
                out[QG * g:QG * (g + 1), :].rearrange(
                    "(a q) d -> q a d", a=4),
                out_sb[:].rearrange("q (a d) -> q a d", a=4))

        for sc in range(NSC):
            prep(sc)
            if sc == 0:
                nc.gpsimd.dma_start(masks_sb[:],
                                    masks.rearrange("j p c -> p j c"))
            if sc % 2 == 1:
                attn((sc - 1) // 2)


# ---------------- host side ----------------

def _perm(p):
    """Permuted-tile -> true-tile index map (32 tiles of 128 rows)."""
    order = []
    for w in range(T // (8 * P)):
        order += [8 * w + p + 2 * a for a in range(4)]
        order += [8 * w + (1 - p) + 2 * a for a in range(4)]
    return np.array(order)


def _masks(p):
    """Multiplicative 0/1 causal masks [8, 128, 512] for the last 8 key
    tiles of every attention group (window-invariant by construction)."""
    m = np.zeros((8, P, QG), np.float32)
    kl = np.arange(P)[:, None]
    ql = np.arange(P)[None, :]
    tri = (kl <= ql)
    for j in range(8):
        for a in range(4):
            blk = m[j, :, a * P:(a + 1) * P]
            if j < 4:
                if j < a:
                    blk[:] = 1.0
                elif j == a:
                    blk[tri] = 1.0
            else:
                mm = j - 4
                if (p == 0 and mm < a) or (p == 1 and mm <= a):
                    blk[:] = 1.0
    return m


_NC_CACHE = []


def _get_nc():
    if not _NC_CACHE:
        _NC_CACHE.append(build_nc())
    return _NC_CACHE[0]


def _run(norm_inputs, Wq, Wk, Wv, **spmd_kwargs):
    nc = _get_nc()
    xf = np.asarray(norm_inputs, np.float32)
    wqf = np.ascontiguousarray(np.asarray(Wq, np.float32))
    wkf = np.ascontiguousarray(np.asarray(Wk, np.float32))
    wvf = np.ascontiguousarray(np.asarray(Wv, np.float32))
    in_maps = []
    for c in range(NCORES):
        b, p = c // 2, c % 2
        xp = np.ascontiguousarray(
            xf[b].reshape(NKT, P, E)[_perm(p)].reshape(T, E))
        in_maps.append({
            "x": xp, "wq": wqf, "wk": wkf, "wv": wvf,
            "masks": _masks(p),
        })
    res = run_bass_kernel_spmd(nc, in_maps, core_ids=list(range(NCORES)),
                               **spmd_kwargs)
    outf = np.empty((B, T, DH), np.float32)
    for c in range(NCORES):
        b, p = c // 2, c % 2
        oc = res.results[c]["out"].reshape(NG, 4, P, DH)
        full = outf[b].reshape(NKT, P, DH)
        for i in range(NG):
            for a in range(4):
                full[8 * i + p + 2 * a] = oc[i, a]
    return outf, res


def kernel(norm_inputs, Wq, Wk, Wv):
    outf, _ = _run(norm_inputs, Wq, Wk, Wv)
    return outf


# revision 26
# speedup vs baseline: 1.7037x; 1.3899x over previous
"""Trainium2 Bass kernel for single-head causal attention.

Problem: nn_AttentionHead (B=4, T=4096, E=1024, DH=128), fp32 inputs/outputs.

  q = x @ Wq; k = x @ Wk; v = x @ Wv            (per batch)
  out = softmax(causal(q k^T / sqrt(DH))) @ v

Sharding: 8 cores = 4 batches x 2 query-shards. The two cores of a batch
pair split the queries at 128-row-tile granularity, interleaved (core p
takes q-tiles p, p+2, p+4, ...) so the causal triangle load-balances.

All 8 cores run ONE uniform SPMD program: the per-core query selection and
the causal structure are folded into host-side data:
  - x rows are permuted per core so that within each window of 8 tiles the
    core's own 4 q-tiles come first ("q-half"), the other parity's after.
  - 8 additive mask tiles [128, 512] (0 / -1e30) encode causal validity of
    the last 8 key-tiles of every attention group vs the group's 4
    query-subtiles (identical for every window by construction).

On-chip dataflow per core (all matmuls in float32r: full PE rate, ~1.5e-4
matmul relative error vs 2.4e-3 for bf16):
  - PE-transpose x chunks -> X^T [e, t] (fp32, rounded to f32r at PSUM
    evacuation), project to K^T/Q^T [d, t] and V^T -> V [k, d].
  - Scores computed transposed: S^T[k, q] = (K^T)^T.T.. = matmul(lhsT=K^T
    tile, rhs=Q^T group), softmax over k (the partition dim):
      exp on ACT (no max-subtraction: scores ~ N(0,1), exp is safe),
      denominator via ones-vector matmul accumulated in PSUM,
      AV as out^T[d,q] = matmul(lhsT=V tile, rhs=P^T) -- no P transposes.
  - Final PE transpose back to [q, d], divide by denominator, DMA out.
"""

import numpy as np

import concourse.bass as bass
import concourse.mybir as mybir
import concourse.tile as tile
from concourse import bacc
from concourse.bass_utils import run_bass_kernel_spmd

P = 128
B, T, E, DH = 4, 4096, 1024, 128
ECH = E // P            # 8 e-chunks
NSC = T // (4 * P)      # 8 super-chunks of 512 rows
NG = NSC // 2           # 4 attention groups of 512 queries per core
NKT = T // P            # 32 key tiles
QG = 4 * P              # 512 queries per group
NCORES = 8
SCALE = 1.0 / np.sqrt(DH)
NEG = -1.0e30

f32 = mybir.dt.float32
f32r = mybir.dt.float32r


def _make_identity(nc, ident):
    nc.gpsimd.memset(ident, 0.0)
    nc.gpsimd.affine_select(
        out=ident, in_=ident,
        compare_op=mybir.AluOpType.not_equal,
        fill=1.0, base=0,
        pattern=[[-1, P]], channel_multiplier=1,
    )


def build_nc():
    nc = bacc.Bacc("TRN2", target_bir_lowering=False, debug=False,
                   num_devices=NCORES)
    x = nc.dram_tensor("x", [T, E], f32, kind="ExternalInput").ap()
    wq = nc.dram_tensor("wq", [E, DH], f32, kind="ExternalInput").ap()
    wk = nc.dram_tensor("wk", [E, DH], f32, kind="ExternalInput").ap()
    wv = nc.dram_tensor("wv", [E, DH], f32, kind="ExternalInput").ap()
    masks = nc.dram_tensor("masks", [8, P, QG], f32, kind="ExternalInput").ap()
    out = nc.dram_tensor("out", [T // 2, DH], f32, kind="ExternalOutput").ap()
    den_scr = nc.dram_tensor("den_scr", [NG, QG], f32, kind="Internal").ap()

    with tile.TileContext(nc) as tc:
        _emit(nc, tc, x, wq, wk, wv, masks, out, den_scr)
    nc.compile()
    return nc


def _emit(nc, tc, x, wq, wk, wv, masks, out, den_scr):
    import contextlib
    ctx = contextlib.ExitStack()
    with ctx:
        const = ctx.enter_context(tc.tile_pool(name="const", bufs=1))
        xin_pool = ctx.enter_context(tc.tile_pool(name="xin", bufs=2))
        xt_pool = ctx.enter_context(tc.tile_pool(name="xt", bufs=2))
        kv_pool = ctx.enter_context(tc.tile_pool(name="kv", bufs=1))
        vtt_pool = ctx.enter_context(tc.tile_pool(name="vtt", bufs=2))
        pt_pool = ctx.enter_context(tc.tile_pool(name="pt", bufs=4))
        osb_pool = ctx.enter_context(tc.tile_pool(name="osb", bufs=2))
        sm_pool = ctx.enter_context(tc.tile_pool(name="sm", bufs=2))
        st_psum = ctx.enter_context(
            tc.tile_pool(name="stp", bufs=4, space="PSUM"))
        scr_psum = ctx.enter_context(
            tc.tile_pool(name="scrp", bufs=2, space="PSUM"))
        avt_psum = ctx.enter_context(
            tc.tile_pool(name="avtp", bufs=1, space="PSUM"))
        den_psum = ctx.enter_context(
            tc.tile_pool(name="denp", bufs=1, space="PSUM"))

        # ---- first x chunk, issued before everything else so the PE can
        # start transposing as early as possible ----
        xin0 = xin_pool.tile([P, 4, E], f32, name="xin")
        for tc_i in range(4):
            nc.sync.dma_start(xin0[:, tc_i, :], x[128 * tc_i:128 * (tc_i + 1), :])

        # ---- constants (weights/masks DMA on the GpSimd queue so they
        # don't delay x chunks on the sync queue) ----
        ident = const.tile([P, P], f32)
        _make_identity(nc, ident)
        ident_r = const.tile([P, P], f32r)
        nc.vector.tensor_copy(ident_r[:], ident[:])
        ones_f = const.tile([P, 1], f32)
        nc.gpsimd.memset(ones_f, 1.0)
        ones_r = const.tile([P, 1], f32r)
        nc.vector.tensor_copy(ones_r[:], ones_f[:])

        # PE warmup: dummy matmuls during the initial DMA wait keep the PE
        # busy so the HAM clock-gate opens to 8/8 before real work starts
        warm = avt_psum.tile([P, QG], f32, tag="avt", name="warm")
        for _ in range(48):
            nc.tensor.matmul(warm[:, :P], ident_r[:], ident_r[:],
                             start=True, stop=True)

        w_r = {}
        for name, wap in (("wk", wk), ("wv", wv), ("wq", wq)):
            wtmp = const.tile([P, ECH, DH], f32, name=f"{name}_tmp")
            nc.sync.dma_start(
                wtmp[:], wap.rearrange("(eo ei) d -> ei eo d", ei=P))
            wr = const.tile([P, ECH, DH], f32r, name=f"{name}_r")
            nc.vector.tensor_copy(wr[:], wtmp[:])
            w_r[name] = wr

        # masks are not needed until the first attention group; emitted
        # late so their DMA doesn't delay the first x chunk
        masks_sb = const.tile([P, 8, QG], f32)

        # persistent K^T [d, k], V [k; kt, d], Q^T [d; group, q]
        kt_sb = kv_pool.tile([P, NKT * P], f32r)
        v_sb = kv_pool.tile([P, NKT * DH], f32r)
        qt_sb = kv_pool.tile([P, NG * QG], f32r)

        def evac(dst, src):
            # PSUM->SBUF evacuations on VectorE (ScalarE is kept exp-only
            # so the attention softmax chain never queues behind copies)
            nc.vector.tensor_copy(dst, src)

        def prep(sc):
            if sc == 0:
                xin = xin0
            else:
                xin = xin_pool.tile([P, 4, E], f32, name="xin")
                for tc_i in range(4):
                    r0 = 512 * sc + 128 * tc_i
                    nc.sync.dma_start(xin[:, tc_i, :], x[r0:r0 + P, :])
            # X^T chunk [e(128 x 8ch), t(512)] in f32r
            xt = xt_pool.tile([P, ECH, 4 * P], f32r, name="xt")
            for tc_i in range(4):
                for half in range(2):
                    tp = scr_psum.tile([P, 4 * P], f32, tag="scr",
                                       name="tp_xt")
                    for e4 in range(4):
                        ec = 4 * half + e4
                        nc.tensor.transpose(
                            tp[:, e4 * P:(e4 + 1) * P],
                            xin[:, tc_i, ec * P:(ec + 1) * P],
                            ident[:])
                    evac(xt[:, 4 * half:4 * half + 4,
                            tc_i * P:(tc_i + 1) * P],
                         tp[:].rearrange("p (a t) -> p a t", a=4))

            # K^T chunk: [d, 512 keys]
            ktp = scr_psum.tile([P, 4 * P], f32, tag="scr", name="ktp")
            for ec in range(ECH):
                nc.tensor.matmul(ktp[:], w_r["wk"][:, ec, :], xt[:, ec, :],
                                 start=(ec == 0), stop=(ec == ECH - 1))
            evac(kt_sb[:, 512 * sc:512 * (sc + 1)], ktp[:])

            # V^T chunk then transpose to V natural [k, d]
            vtp = scr_psum.tile([P, 4 * P], f32, tag="scr", name="vtp")
            for ec in range(ECH):
                nc.tensor.matmul(vtp[:], w_r["wv"][:, ec, :], xt[:, ec, :],
                                 start=(ec == 0), stop=(ec == ECH - 1))
            vtt = vtt_pool.tile([P, 4 * P], f32r, name="vtt")
            evac(vtt[:], vtp[:])
            vnp = scr_psum.tile([P, 4 * P], f32r, tag="scr", name="vnp")
            for kb in range(4):
                nc.tensor.transpose(
                    vnp[:, kb * P:(kb + 1) * P],
                    vtt[:, kb * P:(kb + 1) * P],
                    ident_r[:])
            evac(v_sb[:, 512 * sc:512 * (sc + 1)], vnp[:])

            # Q^T: even super-chunks hold this core's queries
            if sc % 2 == 0:
                qtp = scr_psum.tile([P, 4 * P], f32, tag="scr", name="qtp")
                for ec in range(ECH):
                    nc.tensor.matmul(qtp[:], w_r["wq"][:, ec, :],
                                     xt[:, ec, :],
                                     start=(ec == 0), stop=(ec == ECH - 1))
                g = sc // 2
                evac(qt_sb[:, QG * g:QG * (g + 1)], qtp[:])

        def attn(g):
            nk = 8 * g + 8          # key tiles for this group
            qt_g = qt_sb[:, QG * g:QG * (g + 1)]
            avt = avt_psum.tile([P, QG], f32, tag="avt", name="avt")
            den = den_psum.tile([1, QG], f32, name="den")
            for kt in range(nk):
                st = st_psum.tile([P, QG], f32, name="st")
                nc.tensor.matmul(
                    st[:], kt_sb[:, kt * P:(kt + 1) * P], qt_g,
                    start=True, stop=True)
                pt = pt_pool.tile([P, QG], f32r, name="pt")
                nc.scalar.activation(pt[:], st[:],
                                     mybir.ActivationFunctionType.Exp,
                                     scale=SCALE)
                # multiplicative 0/1 causal mask, applied post-exp so the
                # ST-psum slot is released by exp alone
                j = kt - (nk - 8)
                if j >= 0:
                    nc.vector.tensor_mul(pt[:], pt[:], masks_sb[:, j, :])
                nc.tensor.matmul(
                    avt[:], v_sb[:, kt * P:(kt + 1) * P], pt[:],
                    start=(kt == 0), stop=(kt == nk - 1))
                nc.tensor.matmul(
                    den[:], ones_r[:], pt[:],
                    start=(kt == 0), stop=(kt == nk - 1))

            # epilogue: denominators back in natural q layout via DRAM bounce
            den_sb = sm_pool.tile([1, QG], f32, name="den_sb")
            nc.vector.tensor_copy(den_sb[:], den[:])
            nc.sync.dma_start(den_scr[g:g + 1, :], den_sb[:])
            rsrc = sm_pool.tile([P, 4], f32, name="rsrc")
            nc.sync.dma_start(
                rsrc[:], den_scr[g].rearrange("(a q) -> q a", a=4))
            recip = sm_pool.tile([P, 4], f32, name="recip")
            nc.vector.reciprocal(recip[:], rsrc[:])

            avt_sb = osb_pool.tile([P, QG], f32, name="avt_sb")
            nc.vector.tensor_copy(avt_sb[:], avt[:])
            natp = scr_psum.tile([P, QG], f32, tag="scr", name="natp")
            for a in range(4):
                nc.tensor.transpose(
                    natp[:, a * P:(a + 1) * P],
                    avt_sb[:, a * P:(a + 1) * P],
                    ident[:])
            out_sb = osb_pool.tile([P, QG], f32, name="out_sb")
            for a in range(4):
                nc.vector.tensor_scalar_mul(
                    out_sb[:, a * DH:(a + 1) * DH],
                    natp[:, a * DH:(a + 1) * DH],
                    recip[:, a:a + 1])
            nc.sync.dma_start(
                out[QG * g:QG * (g + 1), :].rearrange(
                    "(a q) d -> q a d", a=4),
                out_sb[:].rearrange("q (a d) -> q a d", a=4))

        for sc in range(NSC):
            prep(sc)
            if sc == 0:
                nc.sync.dma_start(masks_sb[:],
                                  masks.rearrange("j p c -> p j c"))
            if sc % 2 == 1:
                attn((sc - 1) // 2)


# ---------------- host side ----------------

def _perm(p):
    """Permuted-tile -> true-tile index map (32 tiles of 128 rows)."""
    order = []
    for w in range(T // (8 * P)):
        order += [8 * w + p + 2 * a for a in range(4)]
        order += [8 * w + (1 - p) + 2 * a for a in range(4)]
    return np.array(order)


def _masks(p):
    """Multiplicative 0/1 causal masks [8, 128, 512] for the last 8 key
    tiles of every attention group (window-invariant by construction)."""
    m = np.zeros((8, P, QG), np.float32)
    kl = np.arange(P)[:, None]
    ql = np.arange(P)[None, :]
    tri = (kl <= ql)
    for j in range(8):
        for a in range(4):
            blk = m[j, :, a * P:(a + 1) * P]
            if j < 4:
                if j < a:
                    blk[:] = 1.0
                elif j == a:
                    blk[tri] = 1.0
            else:
                mm = j - 4
                if (p == 0 and mm < a) or (p == 1 and mm <= a):
                    blk[:] = 1.0
    return m


_NC_CACHE = []


def _get_nc():
    if not _NC_CACHE:
        _NC_CACHE.append(build_nc())
    return _NC_CACHE[0]


def _run(norm_inputs, Wq, Wk, Wv, **spmd_kwargs):
    nc = _get_nc()
    xf = np.asarray(norm_inputs, np.float32)
    wqf = np.ascontiguousarray(np.asarray(Wq, np.float32))
    wkf = np.ascontiguousarray(np.asarray(Wk, np.float32))
    wvf = np.ascontiguousarray(np.asarray(Wv, np.float32))
    in_maps = []
    for c in range(NCORES):
        b, p = c // 2, c % 2
        xp = np.ascontiguousarray(
            xf[b].reshape(NKT, P, E)[_perm(p)].reshape(T, E))
        in_maps.append({
            "x": xp, "wq": wqf, "wk": wkf, "wv": wvf,
            "masks": _masks(p),
        })
    res = run_bass_kernel_spmd(nc, in_maps, core_ids=list(range(NCORES)),
                               **spmd_kwargs)
    outf = np.empty((B, T, DH), np.float32)
    for c in range(NCORES):
        b, p = c // 2, c % 2
        oc = res.results[c]["out"].reshape(NG, 4, P, DH)
        full = outf[b].reshape(NKT, P, DH)
        for i in range(NG):
            for a in range(4):
                full[8 * i + p + 2 * a] = oc[i, a]
    return outf, res


def kernel(norm_inputs, Wq, Wk, Wv):
    outf, _ = _run(norm_inputs, Wq, Wk, Wv)
    return outf


# revision 27
# speedup vs baseline: 1.7041x; 1.0002x over previous
"""Trainium2 Bass kernel for single-head causal attention
(B=4, T=4096, E=1024, DH=128, fp32), sharded over 8 NeuronCores.

Sharding: 8 cores = 4 batches x 2 query-shards; the two cores of a batch pair each
compute K^T/V for only their own query-parity rows (half the X transposes
and K/V projections), then exchange halves with an intra-pair AllGather
through DRAM.

Own/partner split: attention over a core's OWN key tiles (true parity p)
uses locally-produced kt_own/v_own and never waits for the collective;
attention over the PARTNER key tiles reads kt_par/v_par, distributed from
the AllGather output with a dynamic-offset DMA (the partner's rank block
index 1-p is read from the `psel` input at runtime, keeping the program
core-uniform). All own-half phases are scheduled before any partner-half
phase so the collective's channel-init latency hides behind local work.
The two halves accumulate separate PSUM partials (avt/den), combined on
the VectorE in the epilogue.

x input per core is only its own 2048 query rows (window-ordered).
"""

import numpy as np

import concourse.bass as bass
import concourse.mybir as mybir
import concourse.tile as tile
from concourse import bacc
from concourse.bass_utils import run_bass_kernel_spmd

P = 128
B, T, E, DH = 4, 4096, 1024, 128
ECH = E // P            # 8 e-chunks
NW = T // (8 * P)       # 4 windows of 8 key tiles
NG = NW                 # 4 attention groups of 512 queries per core
NKT = T // P            # 32 key tiles
QG = 4 * P              # 512 queries per group
NCORES = 8
SCALE = 1.0 / np.sqrt(DH)

f32 = mybir.dt.float32
f32r = mybir.dt.float32r
i32 = mybir.dt.int32


def _make_identity(nc, ident):
    nc.gpsimd.memset(ident, 0.0)
    nc.gpsimd.affine_select(
        out=ident, in_=ident,
        compare_op=mybir.AluOpType.not_equal,
        fill=1.0, base=0,
        pattern=[[-1, P]], channel_multiplier=1,
    )


def build_nc():
    nc = bacc.Bacc("TRN2", target_bir_lowering=False, debug=False,
                   num_devices=NCORES)
    x = nc.dram_tensor("x", [T // 2, E], f32, kind="ExternalInput").ap()
    wq = nc.dram_tensor("wq", [E, DH], f32, kind="ExternalInput").ap()
    wk = nc.dram_tensor("wk", [E, DH], f32, kind="ExternalInput").ap()
    wv = nc.dram_tensor("wv", [E, DH], f32, kind="ExternalInput").ap()
    masks = nc.dram_tensor("masks", [8, P, QG], f32, kind="ExternalInput").ap()
    psel = nc.dram_tensor("psel", [1, 1], i32, kind="ExternalInput").ap()
    out = nc.dram_tensor("out", [T // 2, DH], f32, kind="ExternalOutput").ap()
    den_scr = nc.dram_tensor("den_scr", [NG, QG], f32, kind="Internal").ap()

    with tile.TileContext(nc) as tc:
        _emit(nc, tc, x, wq, wk, wv, masks, psel, out, den_scr)
    nc.compile()
    return nc


def _emit(nc, tc, x, wq, wk, wv, masks, psel, out, den_scr):
    import contextlib
    ctx = contextlib.ExitStack()
    with ctx:
        const = ctx.enter_context(tc.tile_pool(name="const", bufs=1))
        xin_pool = ctx.enter_context(tc.tile_pool(name="xin", bufs=2))
        xt_pool = ctx.enter_context(tc.tile_pool(name="xt", bufs=2))
        kv_pool = ctx.enter_context(tc.tile_pool(name="kv", bufs=1))
        vtt_pool = ctx.enter_context(tc.tile_pool(name="vtt", bufs=2))
        pt_pool = ctx.enter_context(tc.tile_pool(name="pt", bufs=6))
        half_pool = ctx.enter_context(tc.tile_pool(name="half", bufs=6))
        osb_pool = ctx.enter_context(tc.tile_pool(name="osb", bufs=2))
        sm_pool = ctx.enter_context(tc.tile_pool(name="sm", bufs=6))
        dram_pool = ctx.enter_context(
            tc.tile_pool(name="ccd", bufs=2, space="DRAM"))
        st_psum = ctx.enter_context(
            tc.tile_pool(name="stp", bufs=4, space="PSUM"))
        scr_psum = ctx.enter_context(
            tc.tile_pool(name="scrp", bufs=2, space="PSUM"))
        avt_psum = ctx.enter_context(
            tc.tile_pool(name="avtp", bufs=1, space="PSUM"))
        den_psum = ctx.enter_context(
            tc.tile_pool(name="denp", bufs=1, space="PSUM"))

        # ---- first x chunk before everything else ----
        xin0 = xin_pool.tile([P, 4, E], f32, name="xin")
        for tc_i in range(4):
            nc.sync.dma_start(xin0[:, tc_i, :],
                              x[128 * tc_i:128 * (tc_i + 1), :])

        # ---- constants ----
        ident = const.tile([P, P], f32)
        _make_identity(nc, ident)
        ident_r = const.tile([P, P], f32r)
        nc.vector.tensor_copy(ident_r[:], ident[:])
        ones_f = const.tile([P, 1], f32)
        nc.gpsimd.memset(ones_f, 1.0)
        ones_r = const.tile([P, 1], f32r)
        nc.vector.tensor_copy(ones_r[:], ones_f[:])

        psel_sb = const.tile([1, 1], i32)
        nc.sync.dma_start(psel_sb[:], psel[:])
        par_idx = nc.values_load(psel_sb[0:1, 0:1], min_val=0, max_val=1,
                                 skip_runtime_bounds_check=True)

        # PE warmup during initial DMA wait
        warm = avt_psum.tile([P, QG], f32, tag="avt", name="warm")
        for _ in range(36):
            nc.tensor.matmul(warm[:, :P], ident_r[:], ident_r[:],
                             start=True, stop=True)

        w_r = {}
        for name, wap in (("wk", wk), ("wv", wv), ("wq", wq)):
            wtmp = const.tile([P, ECH, DH], f32, name=f"{name}_tmp")
            nc.sync.dma_start(
                wtmp[:], wap.rearrange("(eo ei) d -> ei eo d", ei=P))
            wr = const.tile([P, ECH, DH], f32r, name=f"{name}_r")
            nc.vector.tensor_copy(wr[:], wtmp[:])
            w_r[name] = wr

        masks_sb = const.tile([P, 8, QG], f32)

        HW = NKT // 2           # 16 key tiles per half
        kt_own = kv_pool.tile([P, HW * P], f32r)
        v_own = kv_pool.tile([P, HW * P], f32r)
        kt_par = kv_pool.tile([P, HW * P], f32r)
        v_par = kv_pool.tile([P, HW * P], f32r)
        qt_sb = kv_pool.tile([P, NG * QG], f32r)

        def evac(dst, src):
            nc.vector.tensor_copy(dst, src)

        def prep(w):
            if w == 0:
                xin = xin0
            else:
                xin = xin_pool.tile([P, 4, E], f32, name="xin")
                for tc_i in range(4):
                    r0 = 512 * w + 128 * tc_i
                    nc.sync.dma_start(xin[:, tc_i, :], x[r0:r0 + P, :])
            xt = xt_pool.tile([P, ECH, 4 * P], f32r, name="xt")
            for tc_i in range(4):
                for half in range(2):
                    tp = scr_psum.tile([P, 4 * P], f32, tag="scr",
                                       name="tp_xt")
                    for e4 in range(4):
                        ec = 4 * half + e4
                        nc.tensor.transpose(
                            tp[:, e4 * P:(e4 + 1) * P],
                            xin[:, tc_i, ec * P:(ec + 1) * P],
                            ident[:])
                    evac(xt[:, 4 * half:4 * half + 4,
                            tc_i * P:(tc_i + 1) * P],
                         tp[:].rearrange("p (a t) -> p a t", a=4))

            ws = slice(4 * w * P, (4 * w + 4) * P)
            cc_in = dram_pool.tile([P, 2 * QG], f32r, name="cc_in")
            ktp = scr_psum.tile([P, 4 * P], f32, tag="scr", name="ktp")
            for ec in range(ECH):
                nc.tensor.matmul(ktp[:], w_r["wk"][:, ec, :], xt[:, ec, :],
                                 start=(ec == 0), stop=(ec == ECH - 1))
            evac(kt_own[:, ws], ktp[:])
            nc.sync.dma_start(cc_in[:, :QG], kt_own[:, ws])

            vtp = scr_psum.tile([P, 4 * P], f32, tag="scr", name="vtp")
            for ec in range(ECH):
                nc.tensor.matmul(vtp[:], w_r["wv"][:, ec, :], xt[:, ec, :],
                                 start=(ec == 0), stop=(ec == ECH - 1))
            vtt = vtt_pool.tile([P, 4 * P], f32r, name="vtt")
            evac(vtt[:], vtp[:])
            vnp = scr_psum.tile([P, 4 * P], f32r, tag="scr", name="vnp")
            for kb in range(4):
                nc.tensor.transpose(
                    vnp[:, kb * P:(kb + 1) * P],
                    vtt[:, kb * P:(kb + 1) * P],
                    ident_r[:])
            evac(v_own[:, ws], vnp[:])
            nc.sync.dma_start(cc_in[:, QG:], v_own[:, ws])

            # exchange halves within the batch pair; pull the partner's
            # rank block with a runtime (psel) offset
            cc_out = dram_pool.tile([2, P, 2 * QG], f32r, name="cc_out")
            nc.gpsimd.collective_compute(
                "AllGather", mybir.AluOpType.bypass,
                replica_groups=[[0, 1], [2, 3], [4, 5], [6, 7]],
                ins=[cc_in[:]],
                outs=[cc_out[:]],
            )
            nc.sync.dma_start(kt_par[:, ws],
                              cc_out[bass.ds(par_idx, 1), :, 0:QG])
            nc.sync.dma_start(v_par[:, ws],
                              cc_out[bass.ds(par_idx, 1), :, QG:2 * QG])

            # Q^T for this window's 512 own queries
            qtp = scr_psum.tile([P, 4 * P], f32, tag="scr", name="qtp")
            for ec in range(ECH):
                nc.tensor.matmul(qtp[:], w_r["wq"][:, ec, :], xt[:, ec, :],
                                 start=(ec == 0), stop=(ec == ECH - 1))
            evac(qt_sb[:, QG * w:QG * (w + 1)], qtp[:])

        def attn_half(g, par):
            n = 4 * (g + 1)
            src_k = kt_par if par else kt_own
            src_v = v_par if par else v_own
            qt_g = qt_sb[:, QG * g:QG * (g + 1)]
            avt = avt_psum.tile([P, QG], f32, tag="avt", name="avt")
            den = den_psum.tile([1, QG], f32, name="den")
            idx = 0
            for w in range(g + 1):
                for s in range(4):
                    kc = (4 * w + s) * P
                    st = st_psum.tile([P, QG], f32, name="st")
                    nc.tensor.matmul(st[:], src_k[:, kc:kc + P], qt_g,
                                     start=True, stop=True)
                    pt = pt_pool.tile([P, QG], f32r, name="pt")
                    nc.scalar.activation(pt[:], st[:],
                                         mybir.ActivationFunctionType.Exp,
                                         scale=SCALE)
                    if w == g:
                        j = (4 if par else 0) + s
                        nc.vector.tensor_mul(pt[:], pt[:],
                                             masks_sb[:, j, :])
                    nc.tensor.matmul(avt[:], src_v[:, kc:kc + P], pt[:],
                                     start=(idx == 0), stop=(idx == n - 1))
                    nc.tensor.matmul(den[:], ones_r[:], pt[:],
                                     start=(idx == 0), stop=(idx == n - 1))
                    idx += 1
            avt_h = half_pool.tile([P, QG], f32, tag="h",
                                   name=f"avt_h_{g}_{int(par)}")
            nc.scalar.copy(avt_h[:], avt[:])
            den_h = sm_pool.tile([1, QG], f32, tag="dh",
                                 name=f"den_h_{g}_{int(par)}")
            nc.scalar.copy(den_h[:], den[:])
            return avt_h, den_h

        def epilogue(g, own_h, par_h):
            avt_o, den_o = own_h
            avt_p, den_p = par_h
            avt_sb = osb_pool.tile([P, QG], f32, name="avt_sb")
            nc.gpsimd.tensor_add(avt_sb[:], avt_o[:], avt_p[:])
            den_sb = sm_pool.tile([1, QG], f32, tag="dh", name="den_sb")
            nc.gpsimd.tensor_add(den_sb[:], den_o[:], den_p[:])
            nc.sync.dma_start(den_scr[g:g + 1, :], den_sb[:])
            rsrc = sm_pool.tile([P, 4], f32, name="rsrc")
            nc.sync.dma_start(
                rsrc[:], den_scr[g].rearrange("(a q) -> q a", a=4))
            recip = sm_pool.tile([P, 4], f32, name="recip")
            nc.vector.reciprocal(recip[:], rsrc[:])

            natp = scr_psum.tile([P, QG], f32, tag="scr", name="natp")
            for a in range(4):
                nc.tensor.transpose(
                    natp[:, a * P:(a + 1) * P],
                    avt_sb[:, a * P:(a + 1) * P],
                    ident[:])
            out_sb = osb_pool.tile([P, QG], f32, name="out_sb")
            for a in range(4):
                nc.vector.tensor_scalar_mul(
                    out_sb[:, a * DH:(a + 1) * DH],
                    natp[:, a * DH:(a + 1) * DH],
                    recip[:, a:a + 1])
            nc.sync.dma_start(
                out[QG * g:QG * (g + 1), :].rearrange(
                    "(a q) d -> q a d", a=4),
                out_sb[:].rearrange("q (a d) -> q a d", a=4))

        prep(0)
        nc.sync.dma_start(masks_sb[:], masks.rearrange("j p c -> p j c"))
        prep(1)
        own = {}
        own[0] = attn_half(0, False)
        prep(2)
        own[1] = attn_half(1, False)
        prep(3)
        own[2] = attn_half(2, False)
        own[3] = attn_half(3, False)
        for g in range(NG):
            par = attn_half(g, True)
            epilogue(g, own[g], par)


# ---------------- host side ----------------

def _own_tiles(p):
    return np.array([8 * w + p + 2 * a for w in range(NW) for a in range(4)])


def _masks(p):
    """Multiplicative 0/1 causal masks. j<4: own key tile s=j (true offset
    p+2s) vs own query subtile a (true p+2a). j>=4: partner key tile
    s=j-4 (true 1-p+2s) vs own query subtile a."""
    m = np.zeros((8, P, QG), np.float32)
    kl = np.arange(P)[:, None]
    ql = np.arange(P)[None, :]
    tri = (kl <= ql)
    for j in range(8):
        for a in range(4):
            blk = m[j, :, a * P:(a + 1) * P]
            if j < 4:
                if j < a:
                    blk[:] = 1.0
                elif j == a:
                    blk[tri] = 1.0
            else:
                if (j - 4) < a + p:
                    blk[:] = 1.0
    return m


_NC_CACHE = []


def _get_nc():
    if not _NC_CACHE:
        _NC_CACHE.append(build_nc())
    return _NC_CACHE[0]


def _run(norm_inputs, Wq, Wk, Wv, **spmd_kwargs):
    nc = _get_nc()
    xf = np.asarray(norm_inputs, np.float32)
    wqf = np.ascontiguousarray(np.asarray(Wq, np.float32))
    wkf = np.ascontiguousarray(np.asarray(Wk, np.float32))
    wvf = np.ascontiguousarray(np.asarray(Wv, np.float32))
    in_maps = []
    for c in range(NCORES):
        b, p = c // 2, c % 2
        xp = np.ascontiguousarray(
            xf[b].reshape(NKT, P, E)[_own_tiles(p)].reshape(T // 2, E))
        in_maps.append({
            "x": xp, "wq": wqf, "wk": wkf, "wv": wvf,
            "masks": _masks(p),
            "psel": np.array([[1 - p]], np.int32),
        })
    res = run_bass_kernel_spmd(nc, in_maps, core_ids=list(range(NCORES)),
                               **spmd_kwargs)
    outf = np.empty((B, T, DH), np.float32)
    for c in range(NCORES):
        b, p = c // 2, c % 2
        oc = res.results[c]["out"].reshape(NG, 4, P, DH)
        full = outf[b].reshape(NKT, P, DH)
        for i in range(NG):
            for a in range(4):
                full[8 * i + p + 2 * a] = oc[i, a]
    return outf, res


def kernel(norm_inputs, Wq, Wk, Wv):
    outf, _ = _run(norm_inputs, Wq, Wk, Wv)
    return outf
